# revision 23
# baseline (speedup 1.0000x reference)
"""AUGRU (DIEN, nn_DIEN_77326591197307) Trainium2 Bass kernel.

Full-input contract: kernel(**inputs) takes the complete un-sharded arrays
(B=4096, T=200, D=U=64) and returns the full [4096, 64] fp32 output.

Strategy: pure data parallelism over 8 NeuronCores. Rows are sorted by
sequence length (descending) and dealt round-robin to cores, so at step t
only a prefix of columns is still active; the per-step active width w_t
(multiple of 128) is baked into the compiled program. Masked tails are
exactly frozen (alpha*mask folds the mask into the attention weight), so
skipping them is bit-exact, not an approximation.

Per core, a gate-major fp16 scan over T=200 steps with batch on the free
dim (512 columns max):
  S [128, 512] fp16 SBUF persistent = [hT (U on partitions 0:64) ;
                                       xT_t (D on partitions 64:128)]
  per step: PE-transposes drop the x slab into S's x half; one K=128
  matmul produces [z;r] pre-activations (weights pre-scaled by 0.2 and
  biased via ACT) and one produces [xh;rh] in fp32 PSUM; ACT applies
  relu / tanh; DVE does the clip and gating arithmetic with the
  attention update h' = (1-a*z)h + (a*z)hh; alpha*mask arrives
  pre-broadcast from HBM. h state stays in fp16 on-chip for the scan.
"""
import sys
sys.path.insert(0, '/opt/trn_rl_repo')
from contextlib import ExitStack

import numpy as np

import concourse.bass as bass
import concourse.tile as tile
from concourse import bacc, mybir

F16 = mybir.dt.float16
F32 = mybir.dt.float32
Alu = mybir.AluOpType
Act = mybir.ActivationFunctionType

N_CORES = 8
B = 4096
T = 200
D = 64
U = 64
B_LOC = B // N_CORES  # 512
import os
GRP = int(os.environ.get('GRP', '256'))


def _declare_io(nc):
    io = {}
    io["x"] = nc.dram_tensor("x", [B_LOC, T, D], F32, kind="ExternalInput").ap()
    io["at16"] = nc.dram_tensor("at16", [T, 64, B_LOC], F16, kind="ExternalInput").ap()
    io["w_zr"] = nc.dram_tensor("w_zr", [128, 128], F16, kind="ExternalInput").ap()
    io["w_h"] = nc.dram_tensor("w_h", [128, 128], F16, kind="ExternalInput").ap()
    io["ident"] = nc.dram_tensor("ident", [128, 128], F16, kind="ExternalInput").ap()
    io["eyes"] = nc.dram_tensor("eyes", [128, 64], F16, kind="ExternalInput").ap()
    io["b_zr"] = nc.dram_tensor("b_zr", [128, 1], F32, kind="ExternalInput").ap()
    io["b_xh"] = nc.dram_tensor("b_xh", [64, 1], F32, kind="ExternalInput").ap()
    io["out"] = nc.dram_tensor("out", [B_LOC, U], F32, kind="ExternalOutput").ap()
    return io


def _build_kernel(nc, tc, w_list=None):
    """S layout: partitions 0:64 = hT, 64:128 = xT (keeps every DVE op on
    h partition-aligned at base 0; walrus requires matching start
    partitions on tensor_tensor operands)."""
    if w_list is None:
        w_list = [B_LOC] * T
    io = _declare_io(nc)
    ctx = ExitStack()
    with ctx:
        const_pool = ctx.enter_context(tc.tile_pool(name="const", bufs=1))
        state_pool = ctx.enter_context(tc.tile_pool(name="state", bufs=1))
        xin_pool = ctx.enter_context(tc.tile_pool(name="xin", bufs=3))
        work_pool = ctx.enter_context(tc.tile_pool(name="work", bufs=4))
        ps_x = ctx.enter_context(tc.tile_pool(name="ps_x", bufs=2, space="PSUM"))
        ps_zr = ctx.enter_context(tc.tile_pool(name="ps_zr", bufs=2, space="PSUM"))
        ps_h = ctx.enter_context(tc.tile_pool(name="ps_h", bufs=1, space="PSUM"))

        w_zr = const_pool.tile([128, 128], F16)
        nc.sync.dma_start(w_zr[:], io["w_zr"][:])
        w_h = const_pool.tile([128, 128], F16)
        nc.sync.dma_start(w_h[:], io["w_h"][:])
        ident = const_pool.tile([128, 128], F16)
        nc.sync.dma_start(ident[:], io["ident"][:])
        # eyes[0:64] = I64 (output transposes, base 0);
        # eyes[64:128] = I64 (m-inject lhsT, base 64)
        eyes = const_pool.tile([128, 64], F16)
        nc.sync.dma_start(eyes[:], io["eyes"][:])
        b_zr = const_pool.tile([128, 1], F32)
        nc.sync.dma_start(b_zr[:], io["b_zr"][:])
        b_xh = const_pool.tile([64, 1], F32)
        nc.sync.dma_start(b_xh[:], io["b_xh"][:])

        S = state_pool.tile([128, B_LOC], F16)  # [hT ; xT]
        nc.vector.memset(S[:], 0.0)

        x_view = io["x"].rearrange("(c p) t d -> p c t d", p=128)  # [128,4,T,64]

        for t in range(T):
            w = w_list[t]
            nch = w // 128
            phase = t % 2
            if phase == 0:
                x_nat = xin_pool.tile([128, 2 * 256], F16, tag="x_nat")
                nc.gpsimd.dma_start(
                    x_nat[:, 0 : nch * 2 * 64]
                    .rearrange("p (c s d) -> p c s d", c=nch, s=2, d=64),
                    x_view[:, 0:nch, t : t + 2, :],
                )
                abc2 = xin_pool.tile([64, 2 * B_LOC], F16, tag="abc2")
                nc.sync.dma_start(
                    abc2[:].rearrange("p (s b) -> p s b", s=2)[:, :, 0:w],
                    io["at16"][t : t + 2, :, 0:w].rearrange("t p b -> p t b"),
                )
            xv = x_nat[:].rearrange("p (c s d) -> p c s d", c=4, s=2, d=64)

            # transpose x into PSUM rows 64:128 (base partition 64)
            p_xT = ps_x.tile([128, B_LOC], F16, tag="p_xT")
            for c in range(nch):
                nc.tensor.transpose(
                    p_xT[64:128, c * 128 : (c + 1) * 128],
                    xv[:, c, phase, :],
                    ident[:],
                    tile_position=(0, 64),
                )
            for g0 in range(0, w, 256):
                ge = min(g0 + 256, w)
                nc.scalar.copy(S[64:128, g0:ge], p_xT[64:128, g0:ge])

            t_zr = work_pool.tile([128, B_LOC], F16, tag="t_zr")
            hh = work_pool.tile([64, B_LOC], F16, tag="hh")
            q = work_pool.tile([64, B_LOC], F16, tag="q")
            w1 = work_pool.tile([64, B_LOC], F16, tag="w1")
            u = work_pool.tile([64, B_LOC], F16, tag="u")
            v = work_pool.tile([64, B_LOC], F16, tag="v")
            m = work_pool.tile([128, B_LOC], F16, tag="m")
            for g0 in range(0, w, GRP):
                gw = min(g0 + GRP, w) - g0
                gi = g0 // GRP
                cs = slice(g0, g0 + gw)
                p_zr = ps_zr.tile([128, GRP], F32, tag=f"p_zr{gi}")
                p_h = ps_h.tile([128, GRP], F32, tag=f"p_h{gi}")
                ps = slice(0, gw)
                nc.tensor.matmul(
                    p_zr[:, ps], w_zr[:], S[:, cs], start=True, stop=True,
                    skip_group_check=True,
                )
                nc.tensor.matmul(
                    p_h[:, ps], w_h[:], S[:, cs], start=True, stop=False,
                    skip_group_check=True,
                )
                # t_zr = relu(pre*0.2 + 0.5 + bias)  [z rows 0:64, r rows 64:128]
                nc.scalar.activation(t_zr[:, cs], p_zr[:, ps], Act.Relu, bias=b_zr[:])
                # clip z half; r half is clipped inside the STT
                nc.vector.tensor_scalar_min(t_zr[0:64, cs], t_zr[0:64, cs], 1.0)
                # m = min(r', 1) * rh   (zero recurrent h-bias assumed)
                nc.vector.scalar_tensor_tensor(
                    m[64:128, cs], t_zr[64:128, cs], 1.0, p_h[64:128, ps],
                    Alu.min, Alu.mult,
                )
                # p_h[0:64] (xh) += m
                nc.tensor.matmul(
                    p_h[0:64, ps], eyes[64:128, :], m[64:128, cs],
                    start=False, stop=True, skip_group_check=True,
                )
                nc.scalar.activation(hh[:, cs], p_h[0:64, ps], Act.Tanh, bias=b_xh[:])
                abc = abc2[:, phase * B_LOC + cs.start : phase * B_LOC + cs.stop]
                # h' = (1 - a*z) h + (a*z) hh
                nc.vector.tensor_mul(q[:, cs], t_zr[0:64, cs], abc)
                nc.vector.tensor_scalar(w1[:, cs], q[:, cs], -1.0, 1.0, Alu.mult, Alu.add)
                nc.gpsimd.tensor_mul(u[:, cs], w1[:, cs], S[0:64, cs])
                nc.vector.tensor_mul(v[:, cs], q[:, cs], hh[:, cs])
                nc.vector.tensor_add(S[0:64, cs], u[:, cs], v[:, cs])

        # ---- output: transpose hT -> [512, 64] fp32 ----
        h_tmp = work_pool.tile([64, B_LOC], F16, tag="h_tmp")
        nc.vector.tensor_copy(h_tmp[:], S[0:64, :])
        p_out = ps_x.tile([128, 256], F16, tag="p_xT")
        for c in range(4):
            nc.tensor.transpose(
                p_out[:, c * 64 : (c + 1) * 64],
                h_tmp[:, c * 128 : (c + 1) * 128],
                eyes[0:64, :],
            )
        out_sb = work_pool.tile([128, 256], F32, tag="out_sb")
        nc.vector.tensor_copy(out_sb[:], p_out[:])
        nc.sync.dma_start(
            io["out"].rearrange("(c p) d -> p c d", p=128),
            out_sb[:].rearrange("p (c d) -> p c d", c=4),
        )
    return io


_CACHE = {}


def _plan_widths(lengths):
    """Per-step active width per core (multiple of 128), given the global
    sorted-descending round-robin deal."""
    lengths = np.asarray(lengths)
    n_t = (lengths[:, None] > np.arange(T)[None, :]).sum(0)
    per_core = np.ceil(n_t / N_CORES)
    w = (np.ceil(per_core / 128.0) * 128).astype(int)
    w = np.maximum(w, 128)
    w = np.minimum(w, B_LOC)
    w = np.maximum.accumulate(w[::-1])[::-1]  # nonincreasing
    for i in range(0, T - 1, 2):  # x/alpha DMAs cover step pairs
        w[i + 1] = w[i]
    return [int(x) for x in w]


def _get_compiled(w_key):
    if w_key not in _CACHE:
        nc = bacc.Bacc("TRN2", target_bir_lowering=False, num_devices=N_CORES)
        with tile.TileContext(nc) as tc:
            _build_kernel(nc, tc, list(w_key))
        nc.compile()
        _CACHE[w_key] = {"nc": nc}
    return _CACHE[w_key]


def host_prep(inputs, alphas, mask, kernel, recurrent_kernel, bias):
    """Sort rows by length desc, deal round-robin to cores, pack weights.

    Returns (in_maps, w_list, order): order[r] = original row index of
    global sorted rank r (core r % N_CORES, position r // N_CORES).
    """
    Wx = np.asarray(kernel, np.float32)
    Wr = np.asarray(recurrent_kernel, np.float32)
    bias = np.asarray(bias, np.float32)
    b_in, b_rec = bias[0], bias[1]

    # S rows: [h (0:64) ; x (64:128)] -> lhsT rows likewise
    w_zr = np.concatenate([0.2 * Wr[:, :128], 0.2 * Wx[:, :128]], axis=0)
    w_h = np.zeros((128, 128), np.float32)
    w_h[64:128, 0:64] = Wx[:, 128:192]   # x rows -> xh out block
    w_h[0:64, 64:128] = Wr[:, 128:192]   # h rows -> rh out block
    b_zr = (0.2 * (b_in + b_rec)[0:128] + 0.5).reshape(128, 1)
    b_xh = b_in[128:192].reshape(64, 1)
    # the STT clip-fusion assumes the recurrent h-gate bias is zero (true
    # for this model: bias input is zeros)
    assert np.allclose(b_rec[128:192], 0.0), "nonzero recurrent h-bias unsupported"

    mask = np.asarray(mask)
    lengths = mask.sum(1).astype(np.int64)
    order = np.argsort(-lengths, kind="stable")
    w_list = _plan_widths(lengths)

    at = (np.asarray(alphas, np.float32) * mask.astype(np.float32)).T  # [T, B]

    eyes = np.concatenate([np.eye(64, dtype=np.float16)] * 2, axis=0)
    common = {
        "w_zr": w_zr.astype(np.float16),
        "w_h": w_h.astype(np.float16),
        "ident": np.eye(128, dtype=np.float16),
        "eyes": eyes,
        "b_zr": b_zr,
        "b_xh": b_xh,
    }
    x_full = np.asarray(inputs, np.float32)
    in_maps = []
    for c in range(N_CORES):
        rows = order[c::N_CORES]
        mcore = dict(common)
        mcore["x"] = np.ascontiguousarray(x_full[rows])
        atc = np.ascontiguousarray(at[:, rows]).astype(np.float16)  # [T, 512]
        mcore["at16"] = np.ascontiguousarray(
            np.broadcast_to(atc[:, None, :], (T, 64, atc.shape[1]))
        )
        in_maps.append(mcore)
    return in_maps, w_list, order


def _get_executor(w_key):
    """Build (once per width plan) a cached sharded jit callable."""
    entry = _get_compiled(w_key)
    if "exec" in entry:
        return entry["exec"]
    import jax
    from jax.experimental.shard_map import shard_map
    from jax.sharding import Mesh, PartitionSpec
    from concourse import bass2jax, mybir as mb

    nc = entry["nc"]
    bass2jax.install_neuronx_cc_hook()

    partition_name = nc.partition_id_tensor.name if nc.partition_id_tensor else None
    in_names, out_names, out_avals = [], [], []
    for alloc in nc.m.functions[0].allocations:
        if not isinstance(alloc, mb.MemoryLocationSet):
            continue
        name = alloc.memorylocations[0].name
        if alloc.kind == "ExternalInput":
            if name != partition_name:
                in_names.append(name)
        elif alloc.kind == "ExternalOutput":
            out_names.append(name)
            out_avals.append(
                jax.core.ShapedArray(tuple(alloc.tensor_shape), mb.dt.np(alloc.dtype))
            )
    n_params = len(in_names)
    all_in_names = list(in_names) + list(out_names)
    if partition_name is not None:
        all_in_names.append(partition_name)

    def _body(*args):
        operands = list(args)
        if partition_name is not None:
            operands.append(bass2jax.partition_id_tensor())
        outs = bass2jax._bass_exec_p.bind(
            *operands,
            out_avals=tuple(out_avals),
            in_names=tuple(all_in_names),
            out_names=tuple(out_names),
            lowering_input_output_aliases=(),
            sim_require_finite=True,
            sim_require_nnan=True,
            nc=nc,
        )
        return tuple(outs)

    devices = jax.devices()[:N_CORES]
    mesh = Mesh(np.asarray(devices), ("core",))
    n_outs = len(out_names)
    sharded = jax.jit(
        shard_map(
            _body,
            mesh=mesh,
            in_specs=(PartitionSpec("core"),) * (n_params + n_outs),
            out_specs=(PartitionSpec("core"),) * n_outs,
            check_rep=False,
        ),
        donate_argnums=tuple(range(n_params, n_params + n_outs)),
        keep_unused=True,
    )
    entry["exec"] = (sharded, in_names, out_names, out_avals, mesh)
    return entry["exec"]


def _run(in_maps, w_key):
    sharded, in_names, out_names, out_avals, _ = _get_executor(w_key)
    concat_in = [
        np.concatenate([np.asarray(in_maps[c][n]) for c in range(N_CORES)], axis=0)
        for n in in_names
    ]
    concat_zeros = [
        np.zeros((N_CORES * a.shape[0], *a.shape[1:]), a.dtype) for a in out_avals
    ]
    out_arrs = sharded(*concat_in, *concat_zeros)
    return {
        n: np.asarray(out_arrs[i]).reshape(N_CORES, *out_avals[i].shape)
        for i, n in enumerate(out_names)
    }


def bench(in_maps, w_key, iters=8):
    """Time device-side executions with inputs resident on device."""
    import time as _time
    import jax
    from jax.sharding import NamedSharding, PartitionSpec
    sharded, in_names, out_names, out_avals, mesh = _get_executor(w_key)
    sh = NamedSharding(mesh, PartitionSpec("core"))
    dev_in = [
        jax.device_put(
            np.concatenate([np.asarray(in_maps[c][n]) for c in range(N_CORES)], 0), sh
        )
        for n in in_names
    ]
    jax.block_until_ready(dev_in)
    times = []
    for _ in range(iters):
        zeros = [
            jax.device_put(
                np.zeros((N_CORES * a.shape[0], *a.shape[1:]), a.dtype), sh
            )
            for a in out_avals
        ]
        jax.block_until_ready(zeros)
        t0 = _time.time()
        out = sharded(*dev_in, *zeros)
        jax.block_until_ready(out)
        times.append(_time.time() - t0)
    return times


def kernel(inputs, alphas, mask, kernel, recurrent_kernel, bias):
    in_maps, w_list, order = host_prep(
        inputs, alphas, mask, kernel, recurrent_kernel, bias
    )
    outs = _run(in_maps, tuple(w_list))
    res = outs["out"]  # [N_CORES, B_LOC, U]
    out = np.empty((B, U), np.float32)
    for c in range(N_CORES):
        rows = order[c::N_CORES]
        out[rows] = res[c][: len(rows)].astype(np.float32)
    return out


# revision 30
# speedup vs baseline: 186.4039x; 186.4039x over previous
"""AUGRU (DIEN, nn_DIEN_77326591197307) Trainium2 Bass kernel.

Full-input contract: kernel(**inputs) takes the complete un-sharded arrays
(B=4096, T=200, D=U=64) and returns the full [4096, 64] fp32 output.

Strategy: pure data parallelism over 8 NeuronCores. Rows are sorted by
sequence length (descending) and dealt round-robin to cores, so at step t
only a prefix of columns is still active; the per-step active width w_t
(multiple of 128) is baked into the compiled program. Masked tails are
exactly frozen (alpha*mask folds the mask into the attention weight), so
skipping them is bit-exact, not an approximation.

Per core, a gate-major fp16 scan over T=200 steps with batch on the free
dim (512 columns max):
  S [128, 512] fp16 SBUF persistent = [hT (U on partitions 0:64) ;
                                       xT_t (D on partitions 64:128)]
  per step: PE-transposes drop the x slab into S's x half; one K=128
  matmul produces [z;r] pre-activations (weights pre-scaled by 0.2 and
  biased via ACT) and one produces [xh;rh] in fp32 PSUM; ACT applies
  relu / tanh; DVE does the clip and gating arithmetic with the
  attention update h' = (1-a*z)h + (a*z)hh; alpha*mask arrives
  pre-broadcast from HBM. h state stays in fp16 on-chip for the scan.
"""
import sys
sys.path.insert(0, '/opt/trn_rl_repo')
from contextlib import ExitStack

import numpy as np

import concourse.bass as bass
import concourse.tile as tile
from concourse import bacc, mybir

F16 = mybir.dt.float16
F32 = mybir.dt.float32
Alu = mybir.AluOpType
Act = mybir.ActivationFunctionType

N_CORES = 8
B = 4096
T = 200
D = 64
U = 64
B_LOC = B // N_CORES  # 512
import os
GRP = int(os.environ.get('GRP', '256'))


def _declare_io(nc):
    io = {}
    io["x"] = nc.dram_tensor("x", [B_LOC, T, D], F32, kind="ExternalInput").ap()
    io["at16"] = nc.dram_tensor("at16", [T, 64, B_LOC], F16, kind="ExternalInput").ap()
    io["w_zr"] = nc.dram_tensor("w_zr", [128, 128], F16, kind="ExternalInput").ap()
    io["w_h"] = nc.dram_tensor("w_h", [128, 128], F16, kind="ExternalInput").ap()
    io["ident"] = nc.dram_tensor("ident", [128, 128], F16, kind="ExternalInput").ap()
    io["eyes"] = nc.dram_tensor("eyes", [128, 64], F16, kind="ExternalInput").ap()
    io["b_zr"] = nc.dram_tensor("b_zr", [128, 1], F32, kind="ExternalInput").ap()
    io["b_xh"] = nc.dram_tensor("b_xh", [64, 1], F32, kind="ExternalInput").ap()
    io["out"] = nc.dram_tensor("out", [B_LOC, U], F32, kind="ExternalOutput").ap()
    return io


def _build_kernel(nc, tc, w_list=None, t_run=None, repeats=1):
    """S layout: partitions 0:64 = hT, 64:128 = xT (keeps every DVE op on
    h partition-aligned at base 0; walrus requires matching start
    partitions on tensor_tensor operands)."""
    if w_list is None:
        w_list = [B_LOC] * T
    if t_run is None:
        t_run = T
    io = _declare_io(nc)
    ctx = ExitStack()
    with ctx:
        const_pool = ctx.enter_context(tc.tile_pool(name="const", bufs=1))
        state_pool = ctx.enter_context(tc.tile_pool(name="state", bufs=1))
        xin_pool = ctx.enter_context(tc.tile_pool(name="xin", bufs=3))
        work_pool = ctx.enter_context(tc.tile_pool(name="work", bufs=4))
        ps_x = ctx.enter_context(tc.tile_pool(name="ps_x", bufs=2, space="PSUM"))
        ps_zr = ctx.enter_context(tc.tile_pool(name="ps_zr", bufs=2, space="PSUM"))
        ps_h = ctx.enter_context(tc.tile_pool(name="ps_h", bufs=1, space="PSUM"))

        w_zr = const_pool.tile([128, 128], F16)
        nc.sync.dma_start(w_zr[:], io["w_zr"][:])
        w_h = const_pool.tile([128, 128], F16)
        nc.sync.dma_start(w_h[:], io["w_h"][:])
        ident = const_pool.tile([128, 128], F16)
        nc.sync.dma_start(ident[:], io["ident"][:])
        # eyes[0:64] = I64 (output transposes, base 0);
        # eyes[64:128] = I64 (m-inject lhsT, base 64)
        eyes = const_pool.tile([128, 64], F16)
        nc.sync.dma_start(eyes[:], io["eyes"][:])
        b_zr = const_pool.tile([128, 1], F32)
        nc.sync.dma_start(b_zr[:], io["b_zr"][:])
        b_xh = const_pool.tile([64, 1], F32)
        nc.sync.dma_start(b_xh[:], io["b_xh"][:])

        S = state_pool.tile([128, B_LOC], F16)  # [hT ; xT]
        x_view = io["x"].rearrange("(c p) t d -> p c t d", p=128)  # [128,4,T,64]

        for _rep in range(repeats):
          nc.vector.memset(S[:], 0.0)
          for t in range(t_run):
            w = w_list[t]
            nch = w // 128
            phase = t % 2
            if phase == 0:
                x_nat = xin_pool.tile([128, 2 * 256], F16, tag="x_nat")
                nc.gpsimd.dma_start(
                    x_nat[:, 0 : nch * 2 * 64]
                    .rearrange("p (c s d) -> p c s d", c=nch, s=2, d=64),
                    x_view[:, 0:nch, t : t + 2, :],
                )
                abc2 = xin_pool.tile([64, 2 * B_LOC], F16, tag="abc2")
                nc.sync.dma_start(
                    abc2[:].rearrange("p (s b) -> p s b", s=2)[:, :, 0:w],
                    io["at16"][t : t + 2, :, 0:w].rearrange("t p b -> p t b"),
                )
            xv = x_nat[:].rearrange("p (c s d) -> p c s d", c=4, s=2, d=64)

            # transpose x into PSUM rows 64:128 (base partition 64)
            p_xT = ps_x.tile([128, B_LOC], F16, tag="p_xT")
            for c in range(nch):
                nc.tensor.transpose(
                    p_xT[64:128, c * 128 : (c + 1) * 128],
                    xv[:, c, phase, :],
                    ident[:],
                    tile_position=(0, 64),
                )
            for g0 in range(0, w, 256):
                ge = min(g0 + 256, w)
                nc.scalar.copy(S[64:128, g0:ge], p_xT[64:128, g0:ge])

            t_zr = work_pool.tile([128, B_LOC], F16, tag="t_zr")
            hh = work_pool.tile([64, B_LOC], F16, tag="hh")
            q = work_pool.tile([64, B_LOC], F16, tag="q")
            w1 = work_pool.tile([64, B_LOC], F16, tag="w1")
            u = work_pool.tile([64, B_LOC], F16, tag="u")
            v = work_pool.tile([64, B_LOC], F16, tag="v")
            m = work_pool.tile([128, B_LOC], F16, tag="m")
            for g0 in range(0, w, GRP):
                gw = min(g0 + GRP, w) - g0
                gi = g0 // GRP
                cs = slice(g0, g0 + gw)
                p_zr = ps_zr.tile([128, GRP], F32, tag=f"p_zr{gi}")
                p_h = ps_h.tile([128, GRP], F32, tag=f"p_h{gi}")
                ps = slice(0, gw)
                nc.tensor.matmul(
                    p_zr[:, ps], w_zr[:], S[:, cs], start=True, stop=True,
                    skip_group_check=True,
                )
                nc.tensor.matmul(
                    p_h[:, ps], w_h[:], S[:, cs], start=True, stop=False,
                    skip_group_check=True,
                )
                # t_zr = relu(pre*0.2 + 0.5 + bias)  [z rows 0:64, r rows 64:128]
                nc.scalar.activation(t_zr[:, cs], p_zr[:, ps], Act.Relu, bias=b_zr[:])
                # clip z half; r half is clipped inside the STT
                nc.vector.tensor_scalar_min(t_zr[0:64, cs], t_zr[0:64, cs], 1.0)
                # m = min(r', 1) * rh   (zero recurrent h-bias assumed)
                nc.vector.scalar_tensor_tensor(
                    m[64:128, cs], t_zr[64:128, cs], 1.0, p_h[64:128, ps],
                    Alu.min, Alu.mult,
                )
                # p_h[0:64] (xh) += m
                nc.tensor.matmul(
                    p_h[0:64, ps], eyes[64:128, :], m[64:128, cs],
                    start=False, stop=True, skip_group_check=True,
                )
                nc.scalar.activation(hh[:, cs], p_h[0:64, ps], Act.Tanh, bias=b_xh[:])
                abc = abc2[:, phase * B_LOC + cs.start : phase * B_LOC + cs.stop]
                # h' = (1 - a*z) h + (a*z) hh
                nc.gpsimd.tensor_mul(q[:, cs], t_zr[0:64, cs], abc)
                nc.vector.tensor_scalar(w1[:, cs], q[:, cs], -1.0, 1.0, Alu.mult, Alu.add)
                nc.vector.tensor_mul(u[:, cs], w1[:, cs], S[0:64, cs])
                nc.vector.tensor_mul(v[:, cs], q[:, cs], hh[:, cs])
                nc.vector.tensor_add(S[0:64, cs], u[:, cs], v[:, cs])

        # ---- output: transpose hT -> [512, 64] fp32 ----
        h_tmp = work_pool.tile([64, B_LOC], F16, tag="h_tmp")
        nc.vector.tensor_copy(h_tmp[:], S[0:64, :])
        p_out = ps_x.tile([128, 256], F16, tag="p_xT")
        for c in range(4):
            nc.tensor.transpose(
                p_out[:, c * 64 : (c + 1) * 64],
                h_tmp[:, c * 128 : (c + 1) * 128],
                eyes[0:64, :],
            )
        out_sb = work_pool.tile([128, 256], F32, tag="out_sb")
        nc.vector.tensor_copy(out_sb[:], p_out[:])
        nc.sync.dma_start(
            io["out"].rearrange("(c p) d -> p c d", p=128),
            out_sb[:].rearrange("p (c d) -> p c d", c=4),
        )
    return io


_CACHE = {}


def _plan_widths(lengths):
    """Per-step active width per core (multiple of 128), given the global
    sorted-descending round-robin deal."""
    lengths = np.asarray(lengths)
    n_t = (lengths[:, None] > np.arange(T)[None, :]).sum(0)
    per_core = np.ceil(n_t / N_CORES)
    w = (np.ceil(per_core / 128.0) * 128).astype(int)
    w = np.maximum(w, 128)
    w = np.minimum(w, B_LOC)
    w = np.maximum.accumulate(w[::-1])[::-1]  # nonincreasing
    for i in range(0, T - 1, 2):  # x/alpha DMAs cover step pairs
        w[i + 1] = w[i]
    return [int(x) for x in w]


def _get_compiled(w_key, t_run=None, repeats=1):
    key = (w_key, t_run, repeats)
    if key not in _CACHE:
        nc = bacc.Bacc("TRN2", target_bir_lowering=False, num_devices=N_CORES)
        with tile.TileContext(nc) as tc:
            _build_kernel(nc, tc, list(w_key), t_run=t_run, repeats=repeats)
        nc.compile()
        _CACHE[key] = {"nc": nc}
    return _CACHE[key]


def host_prep(inputs, alphas, mask, kernel, recurrent_kernel, bias):
    """Sort rows by length desc, deal round-robin to cores, pack weights.

    Returns (in_maps, w_list, order): order[r] = original row index of
    global sorted rank r (core r % N_CORES, position r // N_CORES).
    """
    Wx = np.asarray(kernel, np.float32)
    Wr = np.asarray(recurrent_kernel, np.float32)
    bias = np.asarray(bias, np.float32)
    b_in, b_rec = bias[0], bias[1]

    # S rows: [h (0:64) ; x (64:128)] -> lhsT rows likewise
    w_zr = np.concatenate([0.2 * Wr[:, :128], 0.2 * Wx[:, :128]], axis=0)
    w_h = np.zeros((128, 128), np.float32)
    w_h[64:128, 0:64] = Wx[:, 128:192]   # x rows -> xh out block
    w_h[0:64, 64:128] = Wr[:, 128:192]   # h rows -> rh out block
    b_zr = (0.2 * (b_in + b_rec)[0:128] + 0.5).reshape(128, 1)
    b_xh = b_in[128:192].reshape(64, 1)
    # the STT clip-fusion assumes the recurrent h-gate bias is zero (true
    # for this model: bias input is zeros)
    assert np.allclose(b_rec[128:192], 0.0), "nonzero recurrent h-bias unsupported"

    mask = np.asarray(mask)
    lengths = mask.sum(1).astype(np.int64)
    order = np.argsort(-lengths, kind="stable")
    w_list = _plan_widths(lengths)

    at = (np.asarray(alphas, np.float32) * mask.astype(np.float32)).T  # [T, B]

    eyes = np.concatenate([np.eye(64, dtype=np.float16)] * 2, axis=0)
    common = {
        "w_zr": w_zr.astype(np.float16),
        "w_h": w_h.astype(np.float16),
        "ident": np.eye(128, dtype=np.float16),
        "eyes": eyes,
        "b_zr": b_zr,
        "b_xh": b_xh,
    }
    x_full = np.asarray(inputs, np.float32)
    in_maps = []
    for c in range(N_CORES):
        rows = order[c::N_CORES]
        mcore = dict(common)
        mcore["x"] = np.ascontiguousarray(x_full[rows])
        atc = np.ascontiguousarray(at[:, rows]).astype(np.float16)  # [T, 512]
        mcore["at16"] = np.ascontiguousarray(
            np.broadcast_to(atc[:, None, :], (T, 64, atc.shape[1]))
        )
        in_maps.append(mcore)
    return in_maps, w_list, order


def _get_executor(w_key, t_run=None, repeats=1):
    """Build (once per width plan) a cached sharded jit callable."""
    entry = _get_compiled(w_key, t_run, repeats)
    if "exec" in entry:
        return entry["exec"]
    import jax
    from jax.experimental.shard_map import shard_map
    from jax.sharding import Mesh, PartitionSpec
    from concourse import bass2jax, mybir as mb

    nc = entry["nc"]
    bass2jax.install_neuronx_cc_hook()

    partition_name = nc.partition_id_tensor.name if nc.partition_id_tensor else None
    in_names, out_names, out_avals = [], [], []
    for alloc in nc.m.functions[0].allocations:
        if not isinstance(alloc, mb.MemoryLocationSet):
            continue
        name = alloc.memorylocations[0].name
        if alloc.kind == "ExternalInput":
            if name != partition_name:
                in_names.append(name)
        elif alloc.kind == "ExternalOutput":
            out_names.append(name)
            out_avals.append(
                jax.core.ShapedArray(tuple(alloc.tensor_shape), mb.dt.np(alloc.dtype))
            )
    n_params = len(in_names)
    all_in_names = list(in_names) + list(out_names)
    if partition_name is not None:
        all_in_names.append(partition_name)

    def _body(*args):
        operands = list(args)
        if partition_name is not None:
            operands.append(bass2jax.partition_id_tensor())
        outs = bass2jax._bass_exec_p.bind(
            *operands,
            out_avals=tuple(out_avals),
            in_names=tuple(all_in_names),
            out_names=tuple(out_names),
            lowering_input_output_aliases=(),
            sim_require_finite=True,
            sim_require_nnan=True,
            nc=nc,
        )
        return tuple(outs)

    devices = jax.devices()[:N_CORES]
    mesh = Mesh(np.asarray(devices), ("core",))
    n_outs = len(out_names)
    sharded = jax.jit(
        shard_map(
            _body,
            mesh=mesh,
            in_specs=(PartitionSpec("core"),) * (n_params + n_outs),
            out_specs=(PartitionSpec("core"),) * n_outs,
            check_rep=False,
        ),
        donate_argnums=tuple(range(n_params, n_params + n_outs)),
        keep_unused=True,
    )
    entry["exec"] = (sharded, in_names, out_names, out_avals, mesh)
    return entry["exec"]


def _run(in_maps, w_key):
    sharded, in_names, out_names, out_avals, _ = _get_executor(w_key)
    concat_in = [
        np.concatenate([np.asarray(in_maps[c][n]) for c in range(N_CORES)], axis=0)
        for n in in_names
    ]
    concat_zeros = [
        np.zeros((N_CORES * a.shape[0], *a.shape[1:]), a.dtype) for a in out_avals
    ]
    out_arrs = sharded(*concat_in, *concat_zeros)
    return {
        n: np.asarray(out_arrs[i]).reshape(N_CORES, *out_avals[i].shape)
        for i, n in enumerate(out_names)
    }


def bench(in_maps, w_key, iters=8, t_run=None, repeats=1):
    """Time device-side executions with inputs resident on device."""
    import time as _time
    import jax
    from jax.sharding import NamedSharding, PartitionSpec
    sharded, in_names, out_names, out_avals, mesh = _get_executor(w_key, t_run, repeats)
    sh = NamedSharding(mesh, PartitionSpec("core"))
    dev_in = [
        jax.device_put(
            np.concatenate([np.asarray(in_maps[c][n]) for c in range(N_CORES)], 0), sh
        )
        for n in in_names
    ]
    jax.block_until_ready(dev_in)
    times = []
    for _ in range(iters):
        zeros = [
            jax.device_put(
                np.zeros((N_CORES * a.shape[0], *a.shape[1:]), a.dtype), sh
            )
            for a in out_avals
        ]
        jax.block_until_ready(zeros)
        t0 = _time.time()
        out = sharded(*dev_in, *zeros)
        jax.block_until_ready(out)
        times.append(_time.time() - t0)
    return times


def kernel(inputs, alphas, mask, kernel, recurrent_kernel, bias):
    in_maps, w_list, order = host_prep(
        inputs, alphas, mask, kernel, recurrent_kernel, bias
    )
    outs = _run(in_maps, tuple(w_list))
    res = outs["out"]  # [N_CORES, B_LOC, U]
    out = np.empty((B, U), np.float32)
    for c in range(N_CORES):
        rows = order[c::N_CORES]
        out[rows] = res[c][: len(rows)].astype(np.float32)
    return out
